# revision 24
# baseline (speedup 1.0000x reference)
"""PointerNet additive-attention scores kernel for Trainium2 (8 NeuronCores).

Math (reference):
    kt[k,n,h] = key[k,n,:] @ w1_w[h,:] + w1_b[h]
    vt[v,n,h] = value[v,n,:] @ w2_w[h,:] + w2_b[h]
    xi[k,v,n] = sum_h v_w[h] * tanh(kt[k,n,h] + vt[v,n,h]) + v_b
    S[k,n]    = sum_v exp(xi[k,v,n]) * mask[v,n];  S==0 -> 1
    out[k,n,v] = xi[k,v,n] - log(S[k,n])

Sharding: data-parallel over batch N (16) across 8 cores, NLOC=2 batch items
per core. Each core computes its (Lk, Lv, 2) slab independently; host slices
inputs / concatenates outputs.

Per-core dataflow (fully unrolled, Tile framework):
  - Host pre-transposes key/value to [n, d, k] and w1/w2 to [d, h] so every
    device DMA is contiguous; input DMAs are spread across 5 engine queues
    so the prologue is fed within ~3 us.
  - PE (fp32): ktT[h,k] / vtT[h,v] per (n, h-chunk); bias added via a c=1
    ones-row matmul into the same PSUM accumulation group.
  - DVE tensor_scalar (bf16 in0/out + per-partition f32 scalar -> 4x mode):
    X[:, k-slice] = ktT_bf + vtT[:, v]  - the (k,v) broadcast add, one
    instruction per v, free dim 128 (the k axis).
  - ACT: wide tanh over [128, 4096] tiles - the roofline engine
    (Lk*Lv*Nloc*H/128 = 65536 lane-cycles @ 1.2 GHz ~ 55 us/core).
  - PE (bf16): xi columns - T_v [128h, 128k] is the *stationary* operand
    (FWL-accelerated LDWEIGHTS), rhs = v_w column [128, 1], giving
    out = psum_xi[:, v] (full 128 partitions, so the 32-strip base-partition
    rule is satisfied); accumulated over the two h-chunks; seeded with v_b
    via a c=1 ones matmul (one seed per PSUM bank: start=True clears
    has_written bank-wide).
  - Epilogue in [k, (n v)] layout: exp -> mask multiply (mask rows
    replicated across partitions with c=1 ones matmuls) -> free-dim reduce
    -> S==0 guard -> log via DVE polynomial (avoids the ~2.7us ACT
    table-set switch to natural_log) -> per-partition subtract -> one
    contiguous DMA out.
"""

import numpy as np

LK, LV, N, D, H = 128, 128, 16, 256, 256
NCORES = 8
NLOC = N // NCORES  # batch items per core
VB = 32  # v-block per X tile -> ACT free dim 4096
NVB = LV // VB

# ln(m) on m in [1, 2]: degree-6 least-squares fit (max err ~1.5e-6).
_LN_COEF = None


def _ln_coef():
    global _LN_COEF
    if _LN_COEF is None:
        xs = np.linspace(1.0, 2.0, 20001)
        _LN_COEF = np.polynomial.Polynomial.fit(xs, np.log(xs), 6).convert().coef
    return _LN_COEF


_CACHE = {}


def _build_program(reps=1):
    from contextlib import ExitStack

    import concourse.bacc as bacc
    import concourse.mybir as mybir
    import concourse.tile as tile

    f32 = mybir.dt.float32
    i32 = mybir.dt.int32
    bf16 = mybir.dt.bfloat16
    AF = mybir.ActivationFunctionType
    ALU = mybir.AluOpType

    nc = bacc.Bacc("TRN2", target_bir_lowering=False, debug=False)

    keyT = nc.dram_tensor("keyT", [NLOC, D, LK], bf16, kind="ExternalInput").ap()
    valT = nc.dram_tensor("valT", [NLOC, D, LV], bf16, kind="ExternalInput").ap()
    w1T = nc.dram_tensor("w1T", [D, H], bf16, kind="ExternalInput").ap()
    w2T = nc.dram_tensor("w2T", [D, H], bf16, kind="ExternalInput").ap()
    b12r = nc.dram_tensor("b12r", [1, H], f32, kind="ExternalInput").ap()
    vwr = nc.dram_tensor("vwr", [1, H], f32, kind="ExternalInput").ap()
    vbrow = nc.dram_tensor("vbrow", [1, NLOC * LV], f32, kind="ExternalInput").ap()
    maskr = nc.dram_tensor("maskr", [NLOC, LV], f32, kind="ExternalInput").ap()
    scores = nc.dram_tensor("scores", [LK, NLOC, LV], f32, kind="ExternalOutput").ap()

    cf = [float(c) for c in _ln_coef()]
    LN2 = float(np.log(2.0))

    with tile.TileContext(nc) as tc, ExitStack() as ctx:
        const = ctx.enter_context(tc.tile_pool(name="const", bufs=1 if reps == 1 else 2))
        ppre = ctx.enter_context(tc.tile_pool(name="ppre", bufs=2, space="PSUM"))
        pacc = ctx.enter_context(tc.tile_pool(name="pacc", bufs=1, space="PSUM"))
        pepi = ctx.enter_context(tc.tile_pool(name="pepi", bufs=1, space="PSUM"))
        xpool = ctx.enter_context(tc.tile_pool(name="xpool", bufs=6))
        tpool = ctx.enter_context(tc.tile_pool(name="tpool", bufs=6))
        epool = ctx.enter_context(tc.tile_pool(name="epool", bufs=2))

        for _rep in range(reps):
            # ---- input loads, spread over DMA queues so prologue feeds fast ----
            keyT_v = keyT.rearrange("n (c p) k -> p n c k", p=128)
            valT_v = valT.rearrange("n (c p) k -> p n c k", p=128)
            keyT_sb = const.tile([128, NLOC, 2, LK], bf16)  # (d%128, n, d//128, k)
            valT_sb = const.tile([128, NLOC, 2, LV], bf16)
            w1T_sb = const.tile([128, 2, H], bf16)  # (d%128, d//128, h)
            w2T_sb = const.tile([128, 2, H], bf16)
            # sync queue: b1 + w1 + key; scalar queue: b2 + w2 + value;
            # gpsimd (SWDGE): the rest of the small tensors
            nc.sync.dma_start(out=w1T_sb, in_=w1T.rearrange("(c p) h -> p c h", p=128))
            nc.scalar.dma_start(
                out=w2T_sb, in_=w2T.rearrange("(c p) h -> p c h", p=128)
            )
            b12_sb = const.tile([1, H], f32)
            nc.sync.dma_start(out=b12_sb, in_=b12r)
            nc.sync.dma_start(out=keyT_sb[:, 0], in_=keyT_v[:, 0])
            nc.scalar.dma_start(out=valT_sb[:, 0], in_=valT_v[:, 0])
            nc.sync.dma_start(out=keyT_sb[:, 1], in_=keyT_v[:, 1])
            nc.scalar.dma_start(out=valT_sb[:, 1], in_=valT_v[:, 1])
            # v_w as per-partition columns [128, hc]
            vwcol_f32 = const.tile([128, 2], f32)
            nc.gpsimd.dma_start(
                out=vwcol_f32, in_=vwr.rearrange("o (c p) -> p (o c)", p=128)
            )
            vb_sb = const.tile([1, NLOC * LV], f32)
            nc.gpsimd.dma_start(out=vb_sb, in_=vbrow)
            mask_sb = []
            for n in range(NLOC):
                m = const.tile([1, LV], f32, tag=f"mask{n}")
                nc.gpsimd.dma_start(out=m, in_=maskr[n : n + 1, :])
                mask_sb.append(m)
            vw_bf = const.tile([128, 2], bf16)

            ones = const.tile([1, 512], f32)
            nc.vector.memset(ones, 1.0)

            # ---- xi accumulator ([128k, n, 128v] packed into one PSUM bank) ----
            xi_t = pacc.tile([LK, NLOC, LV], f32, tag="xi")

            # b12 = (w1_b + w2_b) as per-partition columns [128, 2]: row DMA
            # + two c=1 matmuls (avoids a 128-descriptor strided DMA).
            b12_ps = pepi.tile([128, 2], f32, tag="b12ps")
            for hc in range(2):
                nc.tensor.matmul(
                    out=b12_ps[:, hc : hc + 1],
                    lhsT=b12_sb[:, hc * 128 : (hc + 1) * 128],
                    rhs=ones[:, :1],
                    start=True,
                    stop=True,
                )
            b12c_sb = const.tile([128, 2], f32)
            nc.vector.tensor_copy(b12c_sb, b12_ps)

            # ---- prologue matmuls: ktT/vtT into PSUM per (n, hc) ----
            # The PSUM->SBUF copies are deferred into the main loop so the
            # first TS block isn't queued behind all four (n, hc) copies on
            # the in-order DVE.
            ktT_bf = const.tile([128, NLOC, 2, LK], bf16)  # (h%128, n, hc, k)
            vtT_sb = const.tile([128, NLOC, 2, LV], f32)
            pk_t, pv_t = {}, {}
            for n in range(NLOC):
                for hc in range(2):
                    hsl = slice(hc * 128, (hc + 1) * 128)
                    pk = ppre.tile([128, LK], f32, tag="pk")
                    for dc in range(2):
                        nc.tensor.matmul(
                            out=pk,
                            lhsT=w1T_sb[:, dc, hsl],
                            rhs=keyT_sb[:, n, dc, :],
                            start=(dc == 0),
                            stop=(dc == 1),
                        )
                    pk_t[(n, hc)] = pk

                    pv = ppre.tile([128, LV], f32, tag="pv")
                    for dc in range(2):
                        nc.tensor.matmul(
                            out=pv,
                            lhsT=w2T_sb[:, dc, hsl],
                            rhs=valT_sb[:, n, dc, :],
                            start=(dc == 0),
                            stop=(dc == 1),
                        )
                    pv_t[(n, hc)] = pv

            # seed xi with v_b everywhere (one start=True per bank: start
            # clears has_written bank-wide); emitted after the prologue so PE
            # reaches the kt/vt matmuls first.
            nc.tensor.matmul(
                out=xi_t.rearrange("k n v -> k (n v)"),
                lhsT=ones[:, :LK],
                rhs=vb_sb,
                start=True,
                stop=True,
            )

            # ln() constants for the DVE log (shared by both epilogues)
            c23 = const.tile([128, 1], i32, tag="c23")
            nc.vector.memset(c23, 23)
            cmant = const.tile([128, 1], i32, tag="cmant")
            nc.vector.memset(cmant, 0x007FFFFF)
            cexp1 = const.tile([128, 1], i32, tag="cexp1")
            nc.vector.memset(cexp1, 0x3F800000)

            def epilogue(n):
                # [k, v] layout; S/logS are per-partition columns.
                nc.tensor.matmul(
                    out=pm_t[:, n, :],
                    lhsT=ones[:, :LK],
                    rhs=mask_sb[n],
                    start=True,
                    stop=True,
                )
                e_sb = epool.tile([LK, LV], f32, tag="e")
                nc.scalar.activation(e_sb, xi_t[:, n, :], AF.Exp)
                me = epool.tile([LK, LV], f32, tag="me")
                nc.vector.tensor_tensor(me, e_sb, pm_t[:, n, :], op=ALU.mult)
                S = epool.tile([LK, 1], f32, tag="S")
                nc.vector.reduce_sum(S, me, axis=mybir.AxisListType.X)
                Sg = epool.tile([LK, 1], f32, tag="Sg")
                # Sg = (S == 0 ? 1 : 0) + S  == reference's where(S==0, 1, S)
                nc.vector.scalar_tensor_tensor(
                    out=Sg, in0=S, scalar=0.0, in1=S, op0=ALU.is_equal, op1=ALU.add
                )
                # logS = ln(Sg): exponent/mantissa split + deg-6 poly, all DVE
                # (avoids the ACT natural_log table-set switch).
                xu = Sg.bitcast(i32)
                e_i = epool.tile([LK, 1], i32, tag="e_i")
                nc.vector.tensor_tensor(e_i, xu, c23, op=ALU.logical_shift_right)
                e_f = epool.tile([LK, 1], f32, tag="e_f")
                nc.vector.tensor_copy(e_f, e_i)  # int -> float convert
                m_i = epool.tile([LK, 1], i32, tag="m_i")
                nc.vector.tensor_tensor(m_i, xu, cmant, op=ALU.bitwise_and)
                nc.vector.tensor_tensor(m_i, m_i, cexp1, op=ALU.bitwise_or)
                m = m_i.bitcast(f32)  # mantissa in [1, 2)
                # Estrin: p = (c0+c1 m) + m2*((c2+c3 m) + m2*(c4+c5 m + c6 m2))
                m2 = epool.tile([LK, 1], f32, tag="m2")
                nc.vector.tensor_tensor(m2, m, m, op=ALU.mult)
                u = epool.tile([LK, 1], f32, tag="u")
                nc.vector.tensor_scalar(
                    out=u, in0=m, scalar1=cf[1], scalar2=cf[0], op0=ALU.mult, op1=ALU.add
                )
                vq = epool.tile([LK, 1], f32, tag="vq")
                nc.vector.tensor_scalar(
                    out=vq, in0=m, scalar1=cf[3], scalar2=cf[2], op0=ALU.mult, op1=ALU.add
                )
                w = epool.tile([LK, 1], f32, tag="w")
                nc.vector.tensor_scalar(
                    out=w, in0=m, scalar1=cf[5], scalar2=cf[4], op0=ALU.mult, op1=ALU.add
                )
                w2 = epool.tile([LK, 1], f32, tag="w2")
                nc.vector.scalar_tensor_tensor(
                    out=w2, in0=m2, scalar=cf[6], in1=w, op0=ALU.mult, op1=ALU.add
                )
                q2 = epool.tile([LK, 1], f32, tag="q2")
                nc.vector.scalar_tensor_tensor(
                    out=q2, in0=m2, scalar=1.0, in1=w2, op0=ALU.mult, op1=ALU.mult
                )
                nc.vector.tensor_tensor(q2, q2, vq, op=ALU.add)
                acc = epool.tile([LK, 1], f32, tag="acc")
                nc.vector.scalar_tensor_tensor(
                    out=acc, in0=m2, scalar=1.0, in1=q2, op0=ALU.mult, op1=ALU.mult
                )
                nc.vector.tensor_tensor(acc, acc, u, op=ALU.add)
                esc = epool.tile([LK, 1], f32, tag="esc")
                nc.vector.tensor_scalar(
                    out=esc, in0=e_f, scalar1=LN2, scalar2=-127.0 * LN2,
                    op0=ALU.mult, op1=ALU.add,
                )
                logS = epool.tile([LK, 1], f32, tag="logS")
                nc.vector.tensor_tensor(logS, esc, acc, op=ALU.add)
                sc = epool.tile([LK, LV], f32, tag="sc")
                nc.vector.tensor_scalar_sub(sc, xi_t[:, n, :], logS)
                nc.sync.dma_start(out=scores[:, n, :], in_=sc)

            pm_t = pepi.tile([LK, NLOC, LV], f32, tag="pm")

            # ---- main loop (block sizes ramp at the ends to shrink the
            # pipeline fill and the final PE/epilogue tail) ----
            RAMP_UP = [4, 4, 8, 16, 32, 32, 32]
            RAMP_DN = [32, 32, 32, 16, 8, 8]
            FLAT = [32, 32, 32, 32]
            for n in range(NLOC):
                for hc in range(2):
                    first = n == 0 and hc == 0
                    last = n == NLOC - 1 and hc == 1
                    blocks = RAMP_UP if first else (RAMP_DN if last else FLAT)
                    # deferred prologue copies (both biases fused into vt)
                    nc.vector.tensor_copy(ktT_bf[:, n, hc, :], pk_t[(n, hc)])
                    nc.vector.tensor_scalar_add(
                        vtT_sb[:, n, hc, :], pv_t[(n, hc)], b12c_sb[:, hc : hc + 1]
                    )
                    v0 = 0
                    for blk in blocks:
                        X = xpool.tile([128, blk, LK], bf16, tag="X")
                        for j in range(blk):
                            nc.vector.tensor_scalar_add(
                                X[:, j, :],
                                ktT_bf[:, n, hc, :],
                                vtT_sb[:, n, hc, v0 + j : v0 + j + 1],
                            )
                        if first and v0 == 0:
                            nc.vector.tensor_copy(vw_bf, vwcol_f32)
                        T = tpool.tile([128, blk, LK], bf16, tag="T")
                        nc.scalar.activation(T, X, AF.Tanh)
                        for j in range(blk):
                            nc.tensor.matmul(
                                out=xi_t[:, n, v0 + j : v0 + j + 1],
                                lhsT=T[:, j, :],
                                rhs=vw_bf[:, hc : hc + 1],
                                start=False,
                                stop=(hc == 1),
                                skip_group_check=True,
                            )
                        v0 += blk
                if n == 0:
                    epilogue(0)
            epilogue(NLOC - 1)

    nc.compile()
    return nc


def _get_program(reps=1):
    if reps not in _CACHE:
        _CACHE[reps] = _build_program(reps)
    return _CACHE[reps]


def _make_in_maps(key, value, mask, w1_w, w1_b, w2_w, w2_b, v_w, v_b):
    import ml_dtypes

    bf = ml_dtypes.bfloat16
    key = np.asarray(key, dtype=np.float32)
    value = np.asarray(value, dtype=np.float32)
    mask_f = np.asarray(mask).astype(np.float32)
    w1T_np = np.ascontiguousarray(np.asarray(w1_w, np.float32).T).astype(bf)  # [D, H]
    w2T_np = np.ascontiguousarray(np.asarray(w2_w, np.float32).T).astype(bf)
    b12r_np = (np.asarray(w1_b, np.float32) + np.asarray(w2_b, np.float32)).reshape(
        1, H
    )
    vwr_np = np.asarray(v_w, np.float32).reshape(1, H)
    vb_np = np.full(
        (1, NLOC * LV), np.float32(np.asarray(v_b).reshape(-1)[0]), np.float32
    )

    in_maps = []
    for c in range(NCORES):
        ns = slice(c * NLOC, (c + 1) * NLOC)
        keyT_c = np.ascontiguousarray(key[:, ns, :].transpose(1, 2, 0)).astype(bf)
        valT_c = np.ascontiguousarray(value[:, ns, :].transpose(1, 2, 0)).astype(bf)
        maskr_c = np.ascontiguousarray(mask_f[:, ns].T)  # [NLOC, LV]
        in_maps.append(
            {
                "keyT": keyT_c,
                "valT": valT_c,
                "w1T": w1T_np,
                "w2T": w2T_np,
                "b12r": b12r_np,
                "vwr": vwr_np,
                "vbrow": vb_np,
                "maskr": maskr_c,
            }
        )
    return in_maps


def kernel(**inputs):
    from concourse.bass_utils import run_bass_kernel_spmd

    nc = _get_program()
    in_maps = _make_in_maps(**inputs)
    res = run_bass_kernel_spmd(nc, in_maps, core_ids=list(range(NCORES)))
    out = np.empty((LK, N, LV), np.float32)
    for c in range(NCORES):
        out[:, c * NLOC : (c + 1) * NLOC, :] = res.results[c]["scores"]
    return out
